# revision 6
# baseline (speedup 1.0000x reference)
"""BiLSTM-CRF on 8 Trainium2 NeuronCores — batch-data-parallel, v3.

Contract: kernel(**inputs) takes the FULL unsharded inputs (as produced by
setup_inputs) and returns the full [B, T] int32 tag tensor.

v3 changes vs v2: the whole LSTM pipeline runs in a gates-on-partitions
layout (token order (t, b) per core):
  - recurrence: per step/dir 16 matmuls [128g,128h]x[128h,8b] (moving dim 8)
    instead of 6 matmuls with 512-wide moving dim; gx is DMA'd into SBUF per
    8-step window and injected into PSUM with one identity matmul; the
    per-step h-matmuls accumulate on top.
  - cell ops run on [128, 16..48] tiles (full engine width) instead of
    [8, 256..512] (1/16 width): sigmoid/tanh/mult/add are ~10-20x cheaper.
  - h is produced directly in the next layer's input layout: zero transposes
    in the recurrence.
  - projection/emissions produce transposed gx via stationary weight chunks.
  - viterbi: `pre` computed on ACT (Identity+bias) to unload DVE.
"""

import numpy as np

B, T_FULL, V, E, H, K = 64, 512, 30000, 256, 256, 32
NCORES = 8
BS = B // NCORES
G4 = 4 * H
WIN = 8  # recurrence gx window (steps) = one PSUM bank

WCOMP_OUT = True  # host hi/lo split of W_out (emissions; cheap, keep)

_cache = {}


def _split_hi_lo(w):
    w = np.asarray(w, np.float32)
    m, e = np.frexp(w)
    hi = np.ldexp(np.round(m * 4096.0) / 4096.0, e).astype(np.float32)
    lo = (w.astype(np.float64) - hi).astype(np.float32)
    return hi, lo


def _pack_gates_rows(w):
    """Reorder leading 4H axis from (i,f,g,o) to (i,f,o,g)."""
    i, f, g, o = np.split(np.asarray(w), 4, axis=0)
    return np.concatenate([i, f, o, g], axis=0)


def _build(T_):
    import concourse.bass as bass
    import concourse.tile as tile
    from concourse import bacc, mybir

    f32 = mybir.dt.float32
    f32r = mybir.dt.float32r
    u16 = mybir.dt.uint16
    u32 = mybir.dt.uint32
    i32 = mybir.dt.int32
    AF = mybir.ActivationFunctionType
    OP = mybir.AluOpType
    AP = bass.AP

    ntok = BS * T_
    nwin = T_ // WIN
    NWO = 2 if WCOMP_OUT else 1

    nc = bacc.Bacc(None, target_bir_lowering=False)

    x0T_d = nc.dram_tensor("x0T", [2 * 128, ntok], f32r, kind="ExternalInput")
    wih0_d = nc.dram_tensor("wih0", [2 * 2 * 128, G4], f32r, kind="ExternalInput")
    wih1_d = nc.dram_tensor("wih1", [2 * 4 * 128, G4], f32r, kind="ExternalInput")
    whh0_d = nc.dram_tensor("whh0", [2 * 2 * 128, G4], f32r, kind="ExternalInput")
    whh1_d = nc.dram_tensor("whh1", [2 * 2 * 128, G4], f32r, kind="ExternalInput")
    bcol_d = nc.dram_tensor("bcol", [128, 32], f32, kind="ExternalInput")
    wout_d = nc.dram_tensor("wout", [4 * NWO * 128, K], f32r, kind="ExternalInput")
    bout_d = nc.dram_tensor("bout", [K, 1], f32, kind="ExternalInput")
    trep_d = nc.dram_tensor("trep", [128, K], f32, kind="ExternalInput")
    srep_d = nc.dram_tensor("srep", [128, 1], f32, kind="ExternalInput")
    erep_d = nc.dram_tensor("erep", [128, 1], f32, kind="ExternalInput")
    kidx_d = nc.dram_tensor("kidx", [128, 4], f32r, kind="ExternalInput")
    sel4_d = nc.dram_tensor("sel4", [128, 4], f32r, kind="ExternalInput")
    iden_d = nc.dram_tensor("iden", [128, 128], f32r, kind="ExternalInput")

    tags_d = [nc.dram_tensor(f"tags{c}", [1, 4 * T_], i32, kind="ExternalOutput") for c in range(2)]

    # gx scratch, transposed: [G4, ntok] with token index (t, b)
    gx_d = {(l, d): nc.dram_tensor(f"gx{l}{d}", [G4, ntok], f32r, kind="Internal")
            for l in range(2) for d in range(2)}
    em_d = nc.dram_tensor("em_scr", [K, ntok], f32, kind="Internal")  # (b, t) order

    with tile.TileContext(nc) as tc:
        with (
            tc.tile_pool(name="const", bufs=1) as cpool,
        ):
            whh = {}
            for l, dram in [(0, whh0_d), (1, whh1_d)]:
                for d in range(2):
                    for k in range(2):
                        t_ = cpool.tile([128, G4], f32r, tag=f"whh{l}{d}{k}", name=f"whh{l}{d}{k}")
                        nc.sync.dma_start(t_[:], dram[(d * 2 + k) * 128:(d * 2 + k + 1) * 128, :])
                        whh[(l, d, k)] = t_
            id128 = cpool.tile([128, 128], f32r, tag="id128")
            nc.sync.dma_start(id128[:], iden_d[:])
            bcol = cpool.tile([128, 32], f32, tag="bcol")
            nc.sync.dma_start(bcol[:], bcol_d[:])
            wout = {}
            for k in range(4 * NWO):
                t_ = cpool.tile([128, K], f32r, tag=f"wout{k}", name=f"wout{k}")
                nc.sync.dma_start(t_[:], wout_d[k * 128:(k + 1) * 128, :])
                wout[k] = t_
            bout = cpool.tile([K, 1], f32, tag="bout")
            nc.sync.dma_start(bout[:], bout_d[:])
            trep = cpool.tile([128, K], f32, tag="trep")
            nc.sync.dma_start(trep[:], trep_d[:])
            srep = cpool.tile([128, 1], f32, tag="srep")
            nc.sync.dma_start(srep[:], srep_d[:])
            erep = cpool.tile([128, 1], f32, tag="erep")
            nc.sync.dma_start(erep[:], erep_d[:])
            kidx = cpool.tile([128, 4], f32r, tag="kidx")
            nc.sync.dma_start(kidx[:], kidx_d[:])
            sel4 = cpool.tile([128, 4], f32r, tag="sel4")
            nc.sync.dma_start(sel4[:], sel4_d[:])

            def proj_phase(l, xT_view, nkc):
                """gxT[g, tok] = W_ih.T-chunks @ xT + b, written to gx_d[(l,d)].

                xT_view(k, lo, hi) -> AP [128, hi-lo] of input chunk k.
                """
                wih_dram = wih0_d if l == 0 else wih1_d
                with (
                    tc.tile_pool(name=f"proj{l}", bufs=1) as pp,
                    tc.tile_pool(name=f"projst{l}", bufs=2) as sp_,
                    tc.tile_pool(name=f"projps{l}", bufs=2, space="PSUM") as ppp,
                ):
                    for d in range(2):
                        wih = {}
                        for k in range(nkc):
                            t_ = pp.tile([128, G4], f32r, tag=f"wih{k}", name=f"wih{k}")
                            nc.sync.dma_start(
                                t_[:],
                                wih_dram[(d * nkc + k) * 128:(d * nkc + k + 1) * 128, :],
                            )
                            wih[k] = t_
                        for w in range(ntok // 512):
                            st = sp_.tile([128, 8 * 512], f32r, tag="st")
                            for m in range(8):
                                ps = ppp.tile([128, 512], f32, tag="pj")
                                for k in range(nkc):
                                    nc.tensor.matmul(
                                        ps[:], wih[k][:, m * 128:(m + 1) * 128],
                                        xT_view(k, w * 512, (w + 1) * 512),
                                        start=(k == 0), stop=(k == nkc - 1),
                                    )
                                bc = bcol[:, (l * 2 + d) * 8 + m:(l * 2 + d) * 8 + m + 1]
                                if m % 2 == 0:
                                    nc.scalar.activation(st[:, m * 512:(m + 1) * 512], ps[:],
                                                         AF.Identity, bias=bc)
                                else:
                                    nc.vector.tensor_scalar(st[:, m * 512:(m + 1) * 512], ps[:],
                                                            bc, None, op0=OP.add)
                            eng = nc.sync if w % 2 == 0 else nc.scalar
                            eng.dma_start(
                                AP(gx_d[(l, d)], w * 512,
                                   [[ntok, 128], [128 * ntok, 8], [1, 512]]),
                                st[:])

            def rec_phase(l, xnext):
                """LSTM recurrence in gates-on-partitions layout.

                PSUM window [128, WIN*64]: free = (chunk 8, step WIN, batch 8).
                Gate chunks: 0-1=i, 2-3=f, 4-5=o, 6-7=g (after ifog packing).
                """
                with (
                    tc.tile_pool(name=f"rec{l}", bufs=1) as rp,
                    tc.tile_pool(name=f"recps{l}", bufs=1, space="PSUM") as rpp,
                ):
                    # tgc: [tanh(g) (16) | c (16)] so the t12 mult is ONE op
                    tgc = {d: rp.tile([128, 32], f32, tag=f"tgc{d}", name=f"tgc{d}") for d in range(2)}
                    for d in range(2):
                        nc.vector.memset(tgc[d][:, 16:32], 0.0)
                    xv = {d: xnext[d][:].rearrange("p (k t b) -> p k t b", k=2, b=BS)
                          for d in range(2)}

                    def win_load(d, wi):
                        wt = rp.tile([128, WIN * 64], f32r, tag=f"win{d}", bufs=2, name=f"win{d}")
                        nc.sync.dma_start(
                            wt[:],
                            AP(gx_d[(l, d)], wi * WIN * BS,
                               [[ntok, 128], [128 * ntok, 8], [1, WIN * BS]]),
                        )
                        ps = rpp.tile([128, WIN * 64], f32, tag=f"wps{d}", bufs=2, name=f"wps{d}")
                        nc.tensor.matmul(ps[:], id128[:], wt[:], start=True, stop=False,
                                         skip_group_check=True)
                        return ps

                    win = {0: win_load(0, 0), 1: win_load(1, nwin - 1)}
                    win_next = {}
                    sif = [None, None]
                    t12 = [None, None]
                    tct = [None, None]

                    def sj_of(t, d):
                        s = t if d == 0 else T_ - 1 - t
                        return s, s % WIN

                    def emit_mms(d, t):
                        s, j = sj_of(t, d)
                        wi = s // WIN
                        if t > 0 and j == (0 if d == 0 else WIN - 1):
                            win[d] = win_next[d]
                        if j == (WIN // 2 if d == 0 else WIN // 2 - 1):
                            nwi = wi + 1 if d == 0 else wi - 1
                            if 0 <= nwi < nwin:
                                win_next[d] = win_load(d, nwi)
                        if t == 0:
                            return
                        sp = s - 1 if d == 0 else s + 1
                        ps = win[d]
                        for m in range(8):
                            for k in range(2):
                                nc.tensor.matmul(
                                    ps[:, m * 64 + j * 8:m * 64 + j * 8 + 8],
                                    whh[(l, d, k)][:, m * 128:(m + 1) * 128],
                                    xv[d][:, k, sp, :],
                                    start=False, stop=(k == 1),
                                    skip_group_check=True,
                                )

                    def emit_act(d, t):
                        s, j = sj_of(t, d)
                        pv = win[d][:].rearrange("p (c x) -> p c x", c=8)
                        sif[d] = rp.tile([128, 48], f32, tag=f"sif{d}", bufs=2, name=f"sif{d}")
                        nc.scalar.activation(
                            sif[d][:].rearrange("p (c x) -> p c x", c=6),
                            pv[:, 0:6, j * 8:(j + 1) * 8], AF.Sigmoid)
                        nc.scalar.activation(
                            tgc[d][:, 0:16].rearrange("p (c x) -> p c x", c=2),
                            pv[:, 6:8, j * 8:(j + 1) * 8], AF.Tanh)

                    def emit_cell(d, t):
                        # t12 = (i*tanh_g | f*c); c_new = t12[0:16] + t12[16:32]
                        t12[d] = rp.tile([128, 32], f32, tag=f"t12{d}", bufs=2, name=f"t12{d}")
                        nc.vector.tensor_tensor(t12[d][:], sif[d][:, 0:32], tgc[d][:, 0:32], op=OP.mult)
                        nc.vector.tensor_tensor(tgc[d][:, 16:32], t12[d][:, 0:16],
                                                t12[d][:, 16:32], op=OP.add)

                    def emit_tail(d, t):
                        s, j = sj_of(t, d)
                        tct[d] = rp.tile([128, 16], f32, tag=f"tct{d}", bufs=2, name=f"tct{d}")
                        nc.scalar.activation(tct[d][:], tgc[d][:, 16:32], AF.Tanh)
                        nc.vector.tensor_tensor(
                            xv[d][:, :, s, :],
                            sif[d][:, 32:48].rearrange("p (k b) -> p k b", k=2),
                            tct[d][:].rearrange("p (k b) -> p k b", k=2),
                            op=OP.mult,
                        )

                    # Two direction-chains skewed by one step so each dir's
                    # ACT/DVE block overlaps the other's matmul block.
                    for u in range(T_ + 1):
                        t0_, t1_ = u, u - 1
                        if t0_ < T_:
                            emit_mms(0, t0_)
                        if t1_ >= 0:
                            emit_mms(1, t1_)
                        if t0_ < T_:
                            emit_act(0, t0_)
                            emit_cell(0, t0_)
                        if t1_ >= 0:
                            emit_act(1, t1_)
                        if t0_ < T_:
                            emit_tail(0, t0_)
                        if t1_ >= 0:
                            emit_cell(1, t1_)
                            emit_tail(1, t1_)

            # ================= layers =================
            with tc.tile_pool(name="x1p", bufs=1) as x1pool:
                x1T = {d: x1pool.tile([128, 2 * ntok], f32r, tag=f"x1T{d}", name=f"x1T{d}") for d in range(2)}
                with tc.tile_pool(name="x0p", bufs=1) as x0pool:
                    x0T = [x0pool.tile([128, ntok], f32r, tag=f"x0T{i}", name=f"x0T{i}") for i in range(2)]
                    for k in range(2):
                        nc.sync.dma_start(x0T[k][:], x0T_d[k * 128:(k + 1) * 128, :])
                    proj_phase(0, lambda k, lo, hi: x0T[k][:, lo:hi], 2)
                rec_phase(0, x1T)
                proj_phase(1, lambda k, lo, hi: x1T[k // 2][:, (k % 2) * ntok + lo:(k % 2) * ntok + hi], 4)
            with tc.tile_pool(name="x2p", bufs=1) as x2pool:
                x2T = {d: x2pool.tile([128, 2 * ntok], f32r, tag=f"x2T{d}", name=f"x2T{d}") for d in range(2)}
                rec_phase(1, x2T)
                # ---------- emissions: em[k, (b, t)] ----------
                with (
                    tc.tile_pool(name="emis", bufs=2) as mp,
                    tc.tile_pool(name="emisps", bufs=2, space="PSUM") as mpp,
                ):
                    xr = {d: x2T[d][:].rearrange("p (k t b) -> p k b t", k=2, b=BS)
                          for d in range(2)}
                    for b in range(BS):
                        ps = mpp.tile([K, T_], f32, tag="em")
                        for k in range(4 * NWO):
                            kk = k // NWO
                            nc.tensor.matmul(
                                ps[:], wout[k][:],
                                xr[kk // 2][:, kk % 2, b, :],
                                start=(k == 0), stop=(k == 4 * NWO - 1),
                            )
                        st = mp.tile([K, T_], f32, tag="emst")
                        nc.vector.tensor_scalar(st[:], ps[:], bout[:, 0:1], None, op0=OP.add)
                        nc.sync.dma_start(em_d[:, b * T_:(b + 1) * T_], st[:])
            # ================= viterbi forward =================
            with (
                tc.tile_pool(name="vit", bufs=1) as vp,
                tc.tile_pool(name="vitps", bufs=1, space="PSUM") as vpp,
            ):
                emP, score, bpf32 = {}, {}, {}
                for c in range(2):
                    emP[c] = vp.tile([128, T_], f32, tag=f"emP{c}", name=f"emP{c}")
                    nc.sync.dma_start(
                        emP[c][:],
                        AP(em_d, c * 4 * T_, [[T_, 4], [ntok, 32], [1, T_]]),
                    )
                    score[c] = vp.tile([128, 1], f32, tag=f"score{c}", name=f"score{c}")
                    nc.vector.tensor_tensor(score[c][:], srep[:], emP[c][:, 0:1], op=OP.add)
                    bpf32[c] = vp.tile([128, T_ - 1], f32r, tag=f"bpf{c}", name=f"bpf{c}")
                prev_m8 = {0: None, 1: None}
                # MaxIndex is a leaf (only the backpointer store needs it);
                # defer it one step so the score chain runs back-to-back.
                pend = {}

                def flush_bp(c):
                    pm8, psT, pt = pend[c]
                    bp8 = vp.tile([128, 8], u16, tag=f"bp8{c}", bufs=2, name=f"bp8{c}")
                    nc.vector.max_index(bp8[:], pm8[:], psT[:])
                    nc.scalar.activation(bpf32[c][:, pt - 1:pt], bp8[:, 0:1], AF.Copy)

                for t in range(1, T_):
                    for c in range(2):
                        # pre = trans + score_scalar (+ em_scalar), on ACT via
                        # Identity-with-bias to keep DVE for transpose/max.
                        pre = vp.tile([128, K], f32, tag=f"pre{c}", bufs=2)
                        if t == 1:
                            nc.scalar.activation(pre[:], trep[:], AF.Identity,
                                                 bias=score[c][:, 0:1])
                        else:
                            b2 = vp.tile([128, 1], f32, tag=f"b2{c}", bufs=2)
                            nc.scalar.activation(b2[:], emP[c][:, t - 1:t], AF.Identity,
                                                 bias=prev_m8[c][:, 0:1])
                            nc.scalar.activation(pre[:], trep[:], AF.Identity,
                                                 bias=b2[:, 0:1])
                        sT = vp.tile([128, K], f32, tag=f"sT{c}", bufs=2)
                        nc.vector.transpose(sT[:], pre[:])
                        m8 = vp.tile([128, 8], f32, tag=f"m8{c}", bufs=2)
                        nc.vector.max(m8[:], sT[:])
                        if c in pend:
                            flush_bp(c)
                        pend[c] = (m8, sT, t)
                        prev_m8[c] = m8
                for c in range(2):
                    flush_bp(c)
                # final score + last tag
                fi8, ltf = {}, {}
                for c in range(2):
                    nc.vector.scalar_tensor_tensor(
                        score[c][:], prev_m8[c][:, 0:1], emP[c][:, T_ - 1:T_],
                        erep[:], op0=OP.add, op1=OP.add,
                    )
                    fin = vp.tile([128, K], f32, tag=f"fin{c}")
                    nc.vector.tensor_copy(fin[:], score[c][:, 0:1].to_broadcast([128, K]))
                    finT = vp.tile([128, K], f32, tag=f"finT{c}")
                    nc.vector.transpose(finT[:], fin[:])
                    fm8 = vp.tile([128, 8], f32, tag=f"fm8{c}")
                    fi8[c] = vp.tile([128, 8], u32, tag=f"fi8{c}", name=f"fi8{c}")
                    nc.vector.max(fm8[:], finT[:])
                    nc.vector.max_index(fi8[c][:], fm8[:], finT[:])
                # ================= backtrace =================
                # cur kept as a one-hot column set [128,(i)]; per step ONE matmul
                # with broadcast bp-column as stationary computes
                # cb[p,i] = sum_q bp[q]*onehot[q,i] (= selected tag, bcast to all
                # partitions), then is_equal against kidx rebuilds the one-hot.
                tags1, onehot = {}, {}
                for c in range(2):
                    ltf[c] = vp.tile([128, 1], f32r, tag=f"ltf{c}", name=f"ltf{c}")
                    nc.scalar.activation(ltf[c][:], fi8[c][:, 0:1], AF.Copy)
                    onehot[c] = vp.tile([128, 4], f32r, tag=f"oh{c}", bufs=2, name=f"oh{c}")
                    tags1[c] = vp.tile([1, 4 * T_], f32r, tag=f"tg1{c}", name=f"tg1{c}")

                def step_sel(c, val_col, sel_ap, tcol):
                    cb = vpp.tile([128, 4], f32, tag=f"cb{c}", bufs=2, name=f"cb{c}")
                    nc.tensor.matmul(cb[:], val_col.to_broadcast([128, 128]), sel_ap,
                                     start=True, stop=True)
                    # is_equal first: it feeds the next selection matmul (the
                    # chain); the tags extraction is off-chain on ACT.
                    nc.vector.tensor_tensor(onehot[c][:], kidx[:], cb[:], op=OP.is_equal)
                    nc.scalar.activation(tags1[c][0:1, tcol * 4:(tcol + 1) * 4], cb[0:1, :], AF.Copy)

                for c in range(2):
                    step_sel(c, ltf[c][:, 0:1], sel4[:], T_ - 1)
                for t in range(T_ - 2, -1, -1):
                    for c in range(2):
                        step_sel(c, bpf32[c][:, t:t + 1], onehot[c][:], t)
                for c in range(2):
                    ti = vp.tile([1, 4 * T_], i32, tag=f"ti{c}", name=f"ti{c}")
                    nc.vector.tensor_copy(ti[:], tags1[c][:])
                    nc.sync.dma_start(tags_d[c][:], ti[:])
    nc.compile()
    return nc


def _prep_inputs(inputs, T_):
    """Host preprocessing -> per-core input maps."""
    d = {k: np.asarray(v) for k, v in inputs.items()}
    ids_full = d["inputs"].astype(np.int64)  # [B, T]
    emb = d["emb"].astype(np.float32)

    def wih_pack(l):
        nkc = 2 if l == 0 else 4
        blocks = []
        for dr in ("f", "b"):
            w = _pack_gates_rows(d[f"W_ih_l{l}_{dr}"]).T.astype(np.float32)
            blocks.append(w)
        return np.concatenate(blocks, axis=0)

    def whh_pack(l):
        blocks = []
        for dr in ("f", "b"):
            w = _pack_gates_rows(d[f"W_hh_l{l}_{dr}"]).T.astype(np.float32)
            blocks.append(w)
        return np.concatenate(blocks, axis=0)

    bcol = np.zeros((128, 32), np.float32)
    for l in range(2):
        for di, dr in enumerate(("f", "b")):
            bb = _pack_gates_rows((d[f"b_ih_l{l}_{dr}"] + d[f"b_hh_l{l}_{dr}"]).reshape(4 * H, 1))[:, 0]
            for m in range(8):
                bcol[:, (l * 2 + di) * 8 + m] = bb[m * 128:(m + 1) * 128]

    woutT = d["W_out"].T.astype(np.float32)  # [2H=512, K]
    parts = []
    for k in range(4):
        chunk = woutT[k * 128:(k + 1) * 128, :]
        if WCOMP_OUT:
            hi, lo = _split_hi_lo(chunk)
            parts += [hi, lo]
        else:
            parts += [chunk]
    wout_pack = np.concatenate(parts, axis=0)

    trep = np.tile(d["trans"].astype(np.float32), (4, 1))
    srep = np.tile(d["start_trans"].astype(np.float32), 4).reshape(128, 1)
    erep = np.tile(d["end_trans"].astype(np.float32), 4).reshape(128, 1)
    kidx = np.full((128, 4), -1.0, np.float32)
    sel4 = np.zeros((128, 4), np.float32)
    for i in range(4):
        kidx[i * 32:(i + 1) * 32, i] = np.arange(32, dtype=np.float32)
        sel4[i * 32, i] = 1.0

    common = dict(
        wih0=wih_pack(0), wih1=wih_pack(1),
        whh0=whh_pack(0), whh1=whh_pack(1),
        bcol=bcol, wout=wout_pack,
        bout=d["b_out"].astype(np.float32).reshape(K, 1),
        trep=trep, srep=srep, erep=erep,
        kidx=kidx, sel4=sel4, iden=np.eye(128, dtype=np.float32),
    )
    in_maps = []
    for core in range(NCORES):
        ids_core = ids_full[core * BS:(core + 1) * BS, :T_]  # [BS, T]
        x0 = emb[ids_core]  # [BS, T, E]
        # token order (t, b): x0T[e, t*BS + b]
        x0T = np.ascontiguousarray(x0.transpose(2, 1, 0).reshape(E, BS * T_))
        m = dict(common)
        m["x0T"] = x0T
        in_maps.append(m)
    return in_maps


def _assemble(res, T_):
    tags = np.zeros((B, T_), np.int32)
    for core in range(NCORES):
        r = res[core]
        for c in range(2):
            tags[core * BS + c * 4:core * BS + (c + 1) * 4, :] = (
                r[f"tags{c}"].reshape(T_, 4).T)
    return tags


_staged = {}


def _get_staged(T_):
    """Jitted 8-core staged executable (cached per T_; avoids per-call jax retrace)."""
    if T_ in _staged:
        return _staged[T_]
    import jax
    from jax.sharding import Mesh, PartitionSpec, NamedSharding
    from jax.experimental.shard_map import shard_map
    from concourse import mybir
    from concourse.bass2jax import (
        _bass_exec_p, install_neuronx_cc_hook, partition_id_tensor,
    )

    if T_ not in _cache:
        _cache[T_] = _build(T_)
    nc = _cache[T_]
    install_neuronx_cc_hook()
    partition_name = nc.partition_id_tensor.name if nc.partition_id_tensor else None
    in_names, out_names, out_avals, zero_outs = [], [], [], []
    for alloc in nc.m.functions[0].allocations:
        if not isinstance(alloc, mybir.MemoryLocationSet):
            continue
        name = alloc.memorylocations[0].name
        if alloc.kind == "ExternalInput":
            if name != partition_name:
                in_names.append(name)
        elif alloc.kind == "ExternalOutput":
            shape = tuple(alloc.tensor_shape)
            dtype = mybir.dt.np(alloc.dtype)
            out_names.append(name)
            out_avals.append(jax.core.ShapedArray(shape, dtype))
            zero_outs.append(np.zeros(shape, dtype))
    all_names = list(in_names) + list(out_names)
    if partition_name is not None:
        all_names.append(partition_name)

    def _body(*args):
        operands = list(args)
        if partition_name is not None:
            operands.append(partition_id_tensor())
        return tuple(_bass_exec_p.bind(
            *operands, out_avals=tuple(out_avals), in_names=tuple(all_names),
            out_names=tuple(out_names), lowering_input_output_aliases=(),
            sim_require_finite=True, sim_require_nnan=True, nc=nc))

    devices = jax.devices()[:NCORES]
    mesh = Mesh(np.asarray(devices), ("core",))
    nio = len(in_names) + len(out_names)
    fn = jax.jit(
        shard_map(_body, mesh=mesh,
                  in_specs=(PartitionSpec("core"),) * nio,
                  out_specs=(PartitionSpec("core"),) * len(out_names),
                  check_rep=False),
        keep_unused=True,
    )
    sh = NamedSharding(mesh, PartitionSpec("core"))
    st = dict(fn=fn, sh=sh, in_names=in_names, out_names=out_names,
              zero_outs=zero_outs, jax=jax)
    _staged[T_] = st
    return st


def run(inputs, T_=T_FULL, trace=False):
    if trace:
        from concourse.bass_utils import run_bass_kernel_spmd

        if T_ not in _cache:
            _cache[T_] = _build(T_)
        in_maps = _prep_inputs(inputs, T_)
        res = run_bass_kernel_spmd(_cache[T_], in_maps, core_ids=list(range(NCORES)), trace=trace)
        return _assemble_maps(res.results, T_), res

    st = _get_staged(T_)
    jax = st["jax"]
    in_maps = _prep_inputs(inputs, T_)
    dev_in = [
        jax.device_put(
            np.concatenate([np.asarray(in_maps[c][n]) for c in range(NCORES)], axis=0),
            st["sh"])
        for n in st["in_names"]
    ]
    dev_zero = [
        jax.device_put(np.zeros((NCORES * z.shape[0], *z.shape[1:]), z.dtype), st["sh"])
        for z in st["zero_outs"]
    ]
    outs = st["fn"](*dev_in, *dev_zero)
    res = [
        {name: np.asarray(outs[i]).reshape(NCORES, *st["zero_outs"][i].shape)[c]
         for i, name in enumerate(st["out_names"])}
        for c in range(NCORES)
    ]
    return _assemble_maps(res, T_), None


def _assemble_maps(res, T_):
    return _assemble(res, T_)


def kernel(**inputs):
    tags, _ = run(inputs)
    return tags
